# revision 19
# baseline (speedup 1.0000x reference)
"""Bass/Trainium2 kernel for nn_MultiHeadAttention (T5-style rel-bias causal MHA).

Sharding: 8 cores = 2 batches x 4 head-groups (4 heads of 64 dims each).
Each core: projects q/k/v for its 256 proj rows, runs causal attention, and
computes a partial out-projection. Host sums the 4 partials per batch.

v5 vs v3 (191.6us baseline):
- The T5 relative bias + causal mask fold into a multiplicative exp(band)
  table applied to es on the DVE over a 240-wide near-diagonal stripe
  (exp(s+b) = exp(s)*exp(b); masked positions multiply by 0). This removes
  all PE band-preload matmuls and the const-block bookkeeping; every score
  matmul is a single start/stop K=128 matmul.
- v-projection runs inside the attention phase on the out-projection PSUM
  pool, its 4 l-block groups interleaved with the qs0 attention sections:
  qs0's exp backlog drains on ACT while the PE runs v matmuls. qs0 AV only
  needs v groups 0-1.
- Output DMAs ride the gpsimd ring instead of the ACT ring (ACT paces the
  attention phase via exp).
"""
import math
import sys

sys.path.insert(0, "/opt/trn_rl_repo")

import ml_dtypes
import numpy as np

from concourse import bacc
import concourse.mybir as mybir
import concourse.tile as tile
from concourse.bass_utils import run_bass_kernel_spmd

F32 = mybir.dt.float32
BF16 = mybir.dt.bfloat16
Exp = mybir.ActivationFunctionType.Exp
MUL = mybir.AluOpType.mult
NP_BF16 = ml_dtypes.bfloat16

B, L, D = 2, 2048, 1024
H, HD = 16, 64
NUM_BUCKETS, MAX_DISTANCE = 32, 128
HPC = 4  # heads per core
MPC = HPC * HD  # 256 proj rows per core
N_CORES = 8
BW = 240  # exp-band stripe width (bias==c31 and unmasked beyond it)

last_results = None  # BassKernelResults of the most recent run (for profiling)
_cached = None


def _bucket(rp: np.ndarray) -> np.ndarray:
    """T5 relative position bucket, mirrors the reference exactly."""
    sign = (rp > 0).astype(np.int32)
    n = np.abs(rp)
    max_exact = NUM_BUCKETS // 2
    n_safe = np.maximum(n, 1).astype(np.float32)
    vil = max_exact + (
        np.log(n_safe / max_exact)
        / math.log(MAX_DISTANCE / max_exact)
        * (NUM_BUCKETS - max_exact)
    ).astype(np.int32)
    vil = np.minimum(vil, NUM_BUCKETS - 1)
    buckets = np.where(n < max_exact, n, vil) + sign * max_exact
    return np.clip(buckets, 0, NUM_BUCKETS - 1)


def _build():
    nc = bacc.Bacc(trn_type="TRN2")

    qT_in = nc.dram_tensor("qT_in", [D, L], BF16, kind="ExternalInput")
    kT_in = nc.dram_tensor("kT_in", [D, L], BF16, kind="ExternalInput")
    vT_in = nc.dram_tensor("vT_in", [D, L], BF16, kind="ExternalInput")
    wq_in = nc.dram_tensor("wq_in", [128, 8, MPC], BF16, kind="ExternalInput")
    wk_in = nc.dram_tensor("wk_in", [128, 8, MPC], BF16, kind="ExternalInput")
    wv_in = nc.dram_tensor("wv_in", [128, 8, MPC], BF16, kind="ExternalInput")
    wo_in = nc.dram_tensor("wo_in", [128, 2, D], BF16, kind="ExternalInput")
    bq_in = nc.dram_tensor("bq_in", [128, 2], F32, kind="ExternalInput")
    bk_in = nc.dram_tensor("bk_in", [128, 2], F32, kind="ExternalInput")
    eband_in = nc.dram_tensor("eband_in", [HPC, 128, BW], BF16,
                              kind="ExternalInput")
    outT = nc.dram_tensor("outT", [D, L], BF16, kind="ExternalOutput")

    with tile.TileContext(nc) as tc:
        with (
            tc.tile_pool(name="res", bufs=1) as pr,
            tc.tile_pool(name="qkv", bufs=1) as pqkv,
        ):
            eband_t = [
                pqkv.tile([128, BW], BF16, name=f"eb{hh}") for hh in range(HPC)
            ]
            wo = pqkv.tile([128, 2, D], BF16)
            wv = pqkv.tile([128, 8, MPC], BF16)

            bq = pr.tile([128, 2], F32)
            bk = pr.tile([128, 2], F32)
            # warm the ACT exp table early, off the critical path
            warm = pr.tile([1, 2], F32)
            nc.vector.memset(warm[:], 0.0)
            nc.scalar.activation(warm[:], warm[:], Exp)
            ones_v = pr.tile([1, HD], BF16)
            nc.vector.memset(ones_v[:], 1.0)

            qTz = []
            for hh in range(HPC):
                t = pqkv.tile([128, L], BF16, name=f"qtz{hh}")
                nc.vector.memset(t[:].bitcast(F32), 0.0)
                qTz.append(t)
            kTt = [
                pqkv.tile([128, L], BF16, name=f"kt{mm}") for mm in range(2)
            ]
            vxg = []
            for g in range(4):
                t = pqkv.tile([128, 4, HPC, HD + 1], BF16, name=f"vx{g}")
                nc.vector.memset(t[:, :, :, HD], 1.0)
                vxg.append(t)
            y_norm_qs = [
                pqkv.tile([128, 2, 1024], BF16, name=f"yn{qq}")
                for qq in range(2)
            ]

            # ---------------- q/k projections ----------------
            stgv = []
            with (
                tc.tile_pool(name="proj", bufs=1) as pp,
                tc.tile_pool(name="stg", bufs=16) as pstg,
            ):
                dma_engs = [nc.sync, nc.scalar]
                wq = pp.tile([128, 8, MPC], BF16)
                # split so the kc=0 matmuls can start after ~256KB; the
                # rest rides behind the kc=1 stage
                nc.sync.dma_start(wq[:, 0:2, :], wq_in[:, 0:2, :])
                wk = pp.tile([128, 8, MPC], BF16)

                with tc.tile_pool(name="ppsum", bufs=8, space="PSUM") as pps:
                    # --- q projection: single pass over all 2048 cols ---
                    stgq = []
                    for kc in range(8):
                        stg = pstg.tile([128, L], BF16, tag="stage")
                        if kc == 0:
                            # quarter the first stage across both queues so
                            # the kc=0 n=0 matmul starts ~1us sooner
                            for qtr in range(4):
                                dma_engs[qtr % 2].dma_start(
                                    stg[:, 512 * qtr : 512 * qtr + 512],
                                    qT_in[0:128, 512 * qtr : 512 * qtr + 512],
                                )
                        else:
                            dma_engs[(kc + 1) % 2].dma_start(
                                stg[:], qT_in[128 * kc : 128 * kc + 128, :]
                            )
                        stgq.append(stg)
                        if kc == 1:
                            nc.sync.dma_start(wq[:, 2:8, :], wq_in[:, 2:8, :])
                        if kc == 3:
                            nc.scalar.dma_start(wk[:], wk_in[:])
                        if kc == 7:
                            nc.scalar.dma_start(bq[:], bq_in[:])
                            nc.scalar.dma_start(bk[:], bk_in[:])
                    psums = [
                        pps.tile([128, 512], F32, tag="qk", name=f"qkp{i}")
                        for i in range(8)
                    ]
                    for kc in range(8):
                        for m in range(2):
                            for n in range(4):
                                nc.tensor.matmul(
                                    psums[m * 4 + n][:],
                                    wq[:, kc, 128 * m : 128 * m + 128],
                                    stgq[kc][:, 512 * n : 512 * n + 512],
                                    start=(kc == 0),
                                    stop=(kc == 7),
                                )
                    for m in range(2):
                        for n in range(4):
                            for sub in range(2):
                                pb = 64 * sub
                                nc.vector.tensor_scalar_add(
                                    qTz[2 * m + sub][
                                        pb : pb + 64,
                                        512 * n : 512 * n + 512,
                                    ],
                                    psums[m * 4 + n][pb : pb + 64, :],
                                    bq[pb : pb + 64, m : m + 1],
                                )

                    # --- k projection: two column-half passes, so kT cols
                    # 0-1023 (all qs0 needs) are ready half a projection
                    # early and qs0 scores/exp start sooner ---
                    stgk = []
                    for kc in range(8):
                        stg = pstg.tile([128, L], BF16, tag="stage")
                        dma_engs[kc % 2].dma_start(
                            stg[:], kT_in[128 * kc : 128 * kc + 128, :]
                        )
                        stgk.append(stg)
                        if kc == 5:
                            for hh in range(HPC):
                                nc.scalar.dma_start(
                                    eband_t[hh][:], eband_in[hh]
                                )
                    psk = [
                        pps.tile([128, 512], F32, tag="qk", name=f"kp{i}")
                        for i in range(4)
                    ]
                    for kc in range(8):
                        for m in range(2):
                            for nb in range(2):
                                nc.tensor.matmul(
                                    psk[m * 2 + nb][:],
                                    wk[:, kc, 128 * m : 128 * m + 128],
                                    stgk[kc][:, 512 * nb : 512 * nb + 512],
                                    start=(kc == 0),
                                    stop=(kc == 7),
                                )
                    for m in range(2):
                        for nb in range(2):
                            nc.vector.tensor_scalar_add(
                                kTt[m][:, 512 * nb : 512 * nb + 512],
                                psk[m * 2 + nb][:],
                                bk[:, m : m + 1],
                            )

                    # v stages: loaded here (ring has room once q frees), v
                    # matmuls run in the attention phase below.
                    nc.scalar.dma_start(wv[:], wv_in[:])
                    for kc in range(8):
                        s = pstg.tile([128, L], BF16, tag="stage")
                        dma_engs[kc % 2].dma_start(
                            s[:], vT_in[128 * kc : 128 * kc + 128, :]
                        )
                        stgv.append(s)
                        if kc == 3:
                            nc.sync.dma_start(wo[:], wo_in[:])

                # ---------------- attention + out-projection ----------------
                with (
                    tc.tile_pool(name="es", bufs=12) as pes,
                    tc.tile_pool(name="misc", bufs=3) as pmisc,
                    tc.tile_pool(name="ost", bufs=4) as post,
                    tc.tile_pool(name="spsum", bufs=2, space="PSUM") as psc,
                    tc.tile_pool(name="ypsum", bufs=1, space="PSUM") as psy,
                    tc.tile_pool(name="opsum", bufs=2, space="PSUM") as pso,
                ):
                    def emit_vgrp(grp):
                        # 2 l-blocks at a time on the outproj psum ring
                        for pair in range(2):
                            pv = [
                                pso.tile([128, 512], F32, tag="out",
                                         name=f"vp{grp}{pair}{i}")
                                for i in range(2)
                            ]
                            for kc in range(8):
                                for i in range(2):
                                    li = 4 * grp + 2 * pair + i
                                    nc.tensor.matmul(
                                        pv[i][:, 0:256],
                                        stgv[kc][:, 128 * li : 128 * li + 128],
                                        wv[:, kc, :],
                                        start=(kc == 0),
                                        stop=(kc == 7),
                                    )
                            for i in range(2):
                                li = 4 * grp + 2 * pair + i
                                nc.vector.tensor_copy(
                                    vxg[grp][:, 2 * pair + i, :, 0:HD],
                                    pv[i][:, 0:256].rearrange(
                                        "p (h d) -> p h d", h=HPC
                                    ),
                                )

                    pending_norm = [None]

                    def _emit_norm(item):
                        rrow, pb, mt, qsi = item
                        if qsi == 1:
                            # PE replication: keeps the tail off the gpsimd
                            # ring (busy with output DMAs + broadcasts there)
                            for half in range(2):
                                hof = 512 * half
                                rep = pso.tile([64, 512], F32, tag="out")
                                nc.tensor.matmul(
                                    rep[:],
                                    ones_v[:],
                                    rrow[:, hof : hof + 512],
                                    start=True,
                                    stop=True,
                                )
                                nc.vector.tensor_tensor(
                                    y_norm_qs[qsi][
                                        pb : pb + 64, mt, hof : hof + 512
                                    ],
                                    y_norm_qs[qsi][
                                        pb : pb + 64, mt, hof : hof + 512
                                    ],
                                    rep[:],
                                    MUL,
                                )
                            return
                        prep_sb = pmisc.tile([128, 1024], BF16, tag="prep")
                        nc.gpsimd.partition_broadcast(prep_sb[:], rrow[:])
                        nc.vector.tensor_tensor(
                            y_norm_qs[qsi][pb : pb + 64, mt, :],
                            y_norm_qs[qsi][pb : pb + 64, mt, :],
                            prep_sb[pb : pb + 64, :],
                            MUL,
                        )

                    def _emit_outproj(qsi, qhs=(0, 1)):
                        paired = len(qhs) == 2
                        for n in range(8):
                            ost2 = (
                                post.tile(
                                    [128, 1024], BF16, tag="ost2", name="ost2"
                                )
                                if paired
                                else None
                            )
                            for qh in qhs:
                                qoff = 512 * qh
                                qi = 2 * qsi + qh
                                po = pso.tile([128, 512], F32, tag="out")
                                for c in range(2):
                                    nc.tensor.matmul(
                                        po[:],
                                        wo[:, c, 128 * n : 128 * n + 128],
                                        y_norm_qs[qsi][:, c, qoff : qoff + 512],
                                        start=(c == 0),
                                        stop=(c == 1),
                                    )
                                if paired:
                                    dst = ost2[:, 512 * qh : 512 * qh + 512]
                                else:
                                    dst = post.tile(
                                        [128, 512], BF16, tag="ost", name="ost"
                                    )
                                if qsi == 1 and (n + qh) % 2 == 1:
                                    nc.scalar.copy(dst, po[:])
                                else:
                                    nc.vector.tensor_copy(dst, po[:])
                                if not paired:
                                    [nc.sync, nc.gpsimd][n % 2].dma_start(
                                        outT[
                                            128 * n : 128 * n + 128,
                                            512 * qi : 512 * qi + 512,
                                        ],
                                        dst,
                                    )
                            if paired:
                                # tail (qsi=1) DMAs ride the ACT ring (idle
                                # once exp is done); gpsimd serves qsi=0
                                eng2 = nc.scalar if qsi == 1 else nc.gpsimd
                                [nc.sync, eng2][n % 2].dma_start(
                                    outT[
                                        128 * n : 128 * n + 128,
                                        1024 * qsi : 1024 * qsi + 1024,
                                    ],
                                    ost2[:],
                                )

                    def emit_section(qs, h, defer_av=False):
                        q0 = 1024 * qs
                        n_live = 8 * (qs + 1)
                        mt = h // 2
                        pb = 64 * (h % 2)
                        last = (qs, h) == (1, HPC - 1)
                        live_half = [
                            min(4 * (2 * qs + j + 1), 16) for j in (0, 1)
                        ]
                        yT = psy.tile([HD + 1, 1024], F32, tag="yT")
                        if last and pending_norm[0] is not None:
                            _emit_norm(pending_norm[0])
                            pending_norm[0] = None

                        def _emit_av(pend):
                            pes_t, pspecs, pki = pend
                            for j, sj in pspecs:
                                nc.tensor.matmul(
                                    yT[:, 512 * j + sj : 512 * j + 512],
                                    vxg[pki // 4][:, pki % 4, h, :],
                                    pes_t[:, 512 * j + sj : 512 * j + 512],
                                    start=(pki == 0),
                                    stop=(pki == live_half[j] - 1),
                                )

                        deferred = []
                        pending = None
                        for ki in range(n_live):
                            base = 128 * ki - q0
                            s = max(0, base)
                            specs = [
                                (j, max(0, base - 512 * j))
                                for j in (0, 1)
                                if ki < live_half[j]
                            ]
                            sp = psc.tile([128, 1024], F32, tag="sc",
                                          name="sp")
                            for j, sj in specs:
                                c0 = 512 * j + sj
                                nc.tensor.matmul(
                                    sp[:, c0 : 512 * j + 512],
                                    kTt[mt][:, 128 * ki : 128 * ki + 128],
                                    qTz[h][:, q0 + c0 : q0 + 512 * j + 512],
                                    start=True,
                                    stop=True,
                                )
                            es = pes.tile([128, 1024], BF16, tag="es")
                            nc.scalar.activation(
                                es[:, s:1024], sp[:, s:1024], Exp
                            )
                            e = min(1024, base + BW)
                            if e > s:
                                u0 = s - base
                                nc.vector.tensor_tensor(
                                    es[:, s:e],
                                    es[:, s:e],
                                    eband_t[h][:, u0 : u0 + (e - s)],
                                    MUL,
                                )
                            if defer_av:
                                deferred.append((es, specs, ki))
                            else:
                                if pending is not None:
                                    _emit_av(pending)
                                pending = (es, specs, ki)
                        if defer_av:
                            def flush():
                                for pend in deferred:
                                    _emit_av(pend)
                                _section_tail(qs, h, mt, pb, last, yT)
                            return flush
                        _emit_av(pending)
                        _section_tail(qs, h, mt, pb, last, yT)
                        return None

                    def _section_tail(qs, h, mt, pb, last, yT):
                        if not last:
                            # yT evac FIRST (releases the single yT buffer),
                            # then the recip chain; the replication +
                            # in-place multiply for the PREVIOUS section is
                            # emitted now (its rrow is long ready).
                            nc.vector.tensor_copy(
                                y_norm_qs[qs][pb : pb + 64, mt, :],
                                yT[0:HD, :],
                            )
                            dcp = pmisc.tile([1, 1024], F32, tag="dcp")
                            nc.vector.tensor_copy(dcp[:], yT[HD : HD + 1, :])
                            dT = pmisc.tile([128, 8], F32, tag="dT")
                            nc.sync.dma_start(dT[:], dcp[:])
                            rT = pmisc.tile([128, 8], BF16, tag="rT")
                            with nc.allow_low_precision(
                                reason="softmax recip bf16"
                            ):
                                nc.vector.reciprocal(rT[:], dT[:])
                            rrow = pmisc.tile([1, 1024], BF16, tag="rrow")
                            nc.sync.dma_start(rrow[:], rT[:])
                            if pending_norm[0] is not None:
                                _emit_norm(pending_norm[0])
                            pending_norm[0] = (rrow, pb, mt, qs)
                        else:
                            # final section: lean chain — one custom-DVE
                            # reciprocal straight off the PSUM denominator
                            # row, then fp32 PE replication.
                            # denominator chain on ACT (idle now) so the DVE
                            # yT evacuation doesn't delay the reciprocal
                            dcp = pmisc.tile([1, 1024], F32, tag="dcp")
                            nc.scalar.copy(dcp[:], yT[HD : HD + 1, :])
                            rrec = pmisc.tile([1, 1024], F32, tag="rrec")
                            nc.vector.reciprocal_approx_fast(rrec[:], dcp[:])
                            rrec_b = pmisc.tile([1, 1024], BF16, tag="rrecb")
                            with nc.allow_low_precision(
                                reason="softmax recip bf16"
                            ):
                                nc.scalar.copy(rrec_b[:], rrec[:])
                            nc.vector.tensor_copy(
                                y_norm_qs[qs][pb : pb + 64, mt, :],
                                yT[0:HD, :],
                            )
                            _emit_outproj(0, (1,))
                            for half in range(2):
                                hof = 512 * half
                                rep = pso.tile([64, 512], F32, tag="out")
                                nc.tensor.matmul(
                                    rep[:],
                                    ones_v[:],
                                    rrec_b[:, hof : hof + 512],
                                    start=True,
                                    stop=True,
                                )
                                nc.vector.tensor_tensor(
                                    y_norm_qs[qs][
                                        pb : pb + 64, mt, hof : hof + 512
                                    ],
                                    y_norm_qs[qs][
                                        pb : pb + 64, mt, hof : hof + 512
                                    ],
                                    rep[:],
                                    MUL,
                                )
                        if (qs, h) == (1, 0):
                            _emit_outproj(0, (0,))

                    # Section (0,0)'s scores+exp run as soon as kT cols
                    # 0-1023 exist (k pass A); k pass B then runs on the PE
                    # (psc half-tiles) while qs0's exp backlog drains on ACT.
                    # The v groups interleave with the remaining qs0
                    # sections; qs0 AV only needs groups 0-1.
                    flush00 = emit_section(0, 0, defer_av=True)
                    # k projection pass B: cols 1024-2047 on psc half-tiles
                    # (each 512-f32 half is its own 2KB psum zero-region)
                    pkb = [
                        psc.tile([128, 1024], F32, tag="sc", name=f"kb{m}")
                        for m in range(2)
                    ]
                    for kc in range(8):
                        for m in range(2):
                            for nb in range(2):
                                nc.tensor.matmul(
                                    pkb[m][:, 512 * nb : 512 * nb + 512],
                                    wk[:, kc, 128 * m : 128 * m + 128],
                                    stgk[kc][
                                        :, 1024 + 512 * nb : 1024 + 512 * nb + 512
                                    ],
                                    start=(kc == 0),
                                    stop=(kc == 7),
                                )
                    for m in range(2):
                        nc.vector.tensor_scalar_add(
                            kTt[m][:, 1024:2048],
                            pkb[m][:],
                            bk[:, m : m + 1],
                        )
                    emit_vgrp(0)
                    emit_vgrp(1)
                    flush00()
                    emit_section(0, 1)
                    emit_vgrp(2)
                    emit_section(0, 2)
                    emit_vgrp(3)
                    emit_section(0, 3)
                    for h in range(HPC):
                        emit_section(1, h)
                    _emit_outproj(1)

    nc.finalize()
    return nc


def _host_tables(rel_emb: np.ndarray):
    """Per-head multiplicative exp-band stripes [128, BW]; eb[r,u] multiplies
    es for key-row r, query-col (128*ki + u). Zero above the diagonal
    (causal mask), exp(bias - c31) within distance<113, exactly 1 beyond
    (handled by not multiplying outside the stripe)."""
    r = np.arange(128)[:, None]
    u = np.arange(BW)[None, :]
    rp = r - u  # key - query
    buckets = _bucket(rp)
    ebs = []
    for h in range(H):
        c31 = np.float32(rel_emb[31, h])
        vals = rel_emb[buckets, h].astype(np.float32) - c31
        eb = np.where(rp > 0, np.float32(0.0), np.exp(vals))
        ebs.append(eb.astype(NP_BF16))
    return ebs


def _numpy_ref(query, key, value, attn_mask, key_padding_mask,
               Wq, bq, Wk, bk, Wv, bv, Wo, bo, rel_emb):
    """Exact numpy fallback for unexpected mask patterns."""
    q = (query @ Wq.T + bq).reshape(B, L, H, HD).transpose(0, 2, 1, 3)
    k = (key @ Wk.T + bk).reshape(B, L, H, HD).transpose(0, 2, 1, 3)
    v = (value @ Wv.T + bv).reshape(B, L, H, HD).transpose(0, 2, 1, 3)
    scores = np.einsum("bhqd,bhkd->bhqk", q, k) / math.sqrt(HD)
    rp = np.arange(L, dtype=np.int64)[None, :] - np.arange(L, dtype=np.int64)[:, None]
    rel = rel_emb[_bucket(rp)].transpose(2, 0, 1)
    scores = scores + rel[None]
    scores = np.where(attn_mask[None, None], scores, -np.inf)
    scores = np.where(key_padding_mask[:, None, None, :], scores, -np.inf)
    scores = scores - scores.max(-1, keepdims=True)
    e = np.exp(scores)
    attn = e / e.sum(-1, keepdims=True)
    out = np.einsum("bhqk,bhkd->bhqd", attn, v)
    out = out.transpose(0, 2, 1, 3).reshape(B, L, D)
    return (out @ Wo.T + bo).astype(np.float32)


def _in_maps(inp):
    query, key, value = inp["query"], inp["key"], inp["value"]
    Wq, bq, Wk, bk = inp["Wq"], inp["bq"], inp["Wk"], inp["bk"]
    Wv, Wo = inp["Wv"], inp["Wo"]
    rel_emb = inp["rel_emb"]

    ebands = _host_tables(rel_emb)

    def _rearr_w(w_slice):  # [MPC, D] row-major weights -> [128, 8, MPC]
        arr = np.ascontiguousarray(w_slice.T)  # [D, MPC]
        return arr.reshape(8, 128, MPC).transpose(1, 0, 2).astype(NP_BF16)

    in_maps = []
    for c in range(N_CORES):
        b, hg = c // HPC, c % HPC
        rows = slice(MPC * hg, MPC * hg + MPC)
        heads = range(HPC * hg, HPC * hg + HPC)
        wo_c = np.ascontiguousarray(Wo[:, rows].T)  # [MPC, D]
        in_maps.append({
            "qT_in": query[b].T.astype(NP_BF16),
            "kT_in": key[b].T.astype(NP_BF16),
            "vT_in": value[b].T.astype(NP_BF16),
            "wq_in": _rearr_w(Wq[rows] / math.sqrt(HD)),
            "wk_in": _rearr_w(Wk[rows]),
            "wv_in": _rearr_w(Wv[rows]),
            "wo_in": wo_c.reshape(2, 128, D).transpose(1, 0, 2).astype(NP_BF16),
            "bq_in": np.ascontiguousarray(
                (bq[rows] / math.sqrt(HD)).reshape(2, 128).T.astype(np.float32)
            ),
            "bk_in": np.ascontiguousarray(
                bk[rows].reshape(2, 128).T.astype(np.float32)
            ),
            "eband_in": np.stack([ebands[h] for h in heads]),
        })
    return in_maps


def kernel(**inputs) -> np.ndarray:
    global _cached, last_results
    inp = {k: np.asarray(v) for k, v in inputs.items()}
    attn_mask, kpm = inp["attn_mask"], inp["key_padding_mask"]

    causal = np.array_equal(attn_mask, np.tril(np.ones((L, L), bool)))
    if not (causal and kpm.all()):
        return _numpy_ref(**inp)

    if _cached is None:
        _cached = _build()
    nc = _cached

    res = run_bass_kernel_spmd(nc, _in_maps(inp), list(range(N_CORES)))
    last_results = res

    bo, bv, Wo = inp["bo"], inp["bv"], inp["Wo"]
    bo_eff = (
        bo.astype(np.float64) + bv.astype(np.float64) @ Wo.T.astype(np.float64)
    )
    out = np.empty((B, L, D), np.float32)
    for b in range(B):
        acc = np.zeros((D, L), np.float64)
        for hg in range(HPC):
            acc += res.results[b * HPC + hg]["outT"].astype(np.float64)
        out[b] = (acc.T + bo_eff[None, :]).astype(np.float32)
    return out


# revision 24
# speedup vs baseline: 1.0129x; 1.0129x over previous
"""Bass/Trainium2 kernel for nn_MultiHeadAttention (T5-style rel-bias causal MHA).

Sharding: 8 cores = 2 batches x 4 head-groups (4 heads of 64 dims each).
Each core: projects q/k/v for its 256 proj rows, runs causal attention, and
computes a partial out-projection. Host sums the 4 partials per batch.

v5 vs v3 (191.6us baseline):
- The T5 relative bias + causal mask fold into a multiplicative exp(band)
  table applied to es on the DVE over a 240-wide near-diagonal stripe
  (exp(s+b) = exp(s)*exp(b); masked positions multiply by 0). This removes
  all PE band-preload matmuls and the const-block bookkeeping; every score
  matmul is a single start/stop K=128 matmul.
- v-projection runs inside the attention phase on the out-projection PSUM
  pool, its 4 l-block groups interleaved with the qs0 attention sections:
  qs0's exp backlog drains on ACT while the PE runs v matmuls. qs0 AV only
  needs v groups 0-1.
- Output DMAs ride the gpsimd ring instead of the ACT ring (ACT paces the
  attention phase via exp).
"""
import math
import sys

sys.path.insert(0, "/opt/trn_rl_repo")

import ml_dtypes
import numpy as np

from concourse import bacc
import concourse.mybir as mybir
import concourse.tile as tile
from concourse.bass_utils import run_bass_kernel_spmd

F32 = mybir.dt.float32
BF16 = mybir.dt.bfloat16
Exp = mybir.ActivationFunctionType.Exp
MUL = mybir.AluOpType.mult
NP_BF16 = ml_dtypes.bfloat16

B, L, D = 2, 2048, 1024
H, HD = 16, 64
NUM_BUCKETS, MAX_DISTANCE = 32, 128
HPC = 4  # heads per core
MPC = HPC * HD  # 256 proj rows per core
N_CORES = 8
BW = 240  # exp-band stripe width (bias==c31 and unmasked beyond it)

last_results = None  # BassKernelResults of the most recent run (for profiling)
_cached = None


def _bucket(rp: np.ndarray) -> np.ndarray:
    """T5 relative position bucket, mirrors the reference exactly."""
    sign = (rp > 0).astype(np.int32)
    n = np.abs(rp)
    max_exact = NUM_BUCKETS // 2
    n_safe = np.maximum(n, 1).astype(np.float32)
    vil = max_exact + (
        np.log(n_safe / max_exact)
        / math.log(MAX_DISTANCE / max_exact)
        * (NUM_BUCKETS - max_exact)
    ).astype(np.int32)
    vil = np.minimum(vil, NUM_BUCKETS - 1)
    buckets = np.where(n < max_exact, n, vil) + sign * max_exact
    return np.clip(buckets, 0, NUM_BUCKETS - 1)


def _build():
    nc = bacc.Bacc(trn_type="TRN2")

    qT_in = nc.dram_tensor("qT_in", [D, L], BF16, kind="ExternalInput")
    kT_in = nc.dram_tensor("kT_in", [D, L], BF16, kind="ExternalInput")
    vT_in = nc.dram_tensor("vT_in", [D, L], BF16, kind="ExternalInput")
    wq_in = nc.dram_tensor("wq_in", [128, 8, MPC], BF16, kind="ExternalInput")
    wk_in = nc.dram_tensor("wk_in", [128, 8, MPC], BF16, kind="ExternalInput")
    wv_in = nc.dram_tensor("wv_in", [128, 8, MPC], BF16, kind="ExternalInput")
    wo_in = nc.dram_tensor("wo_in", [128, 2, D], BF16, kind="ExternalInput")
    bq_in = nc.dram_tensor("bq_in", [128, 2], F32, kind="ExternalInput")
    bk_in = nc.dram_tensor("bk_in", [128, 2], F32, kind="ExternalInput")
    eband_in = nc.dram_tensor("eband_in", [HPC, 128, BW], BF16,
                              kind="ExternalInput")
    outT = nc.dram_tensor("outT", [D, L], BF16, kind="ExternalOutput")

    with tile.TileContext(nc) as tc:
        with (
            tc.tile_pool(name="res", bufs=1) as pr,
            tc.tile_pool(name="qkv", bufs=1) as pqkv,
        ):
            eband_t = [
                pqkv.tile([128, BW], BF16, name=f"eb{hh}") for hh in range(HPC)
            ]
            wo = pqkv.tile([128, 2, D], BF16)
            wv = pqkv.tile([128, 8, MPC], BF16)

            bq = pr.tile([128, 2], F32)
            bk = pr.tile([128, 2], F32)
            # warm the ACT exp table early, off the critical path
            warm = pr.tile([1, 2], F32)
            nc.vector.memset(warm[:], 0.0)
            nc.scalar.activation(warm[:], warm[:], Exp)
            ones_v = pr.tile([1, HD], BF16)
            nc.vector.memset(ones_v[:], 1.0)

            qTz = []
            for hh in range(HPC):
                t = pqkv.tile([128, L], BF16, name=f"qtz{hh}")
                nc.vector.memset(t[:].bitcast(F32), 0.0)
                qTz.append(t)
            kTt = [
                pqkv.tile([128, L], BF16, name=f"kt{mm}") for mm in range(2)
            ]
            vxg = []
            for g in range(4):
                t = pqkv.tile([128, 4, HPC, HD + 1], BF16, name=f"vx{g}")
                nc.vector.memset(t[:, :, :, HD], 1.0)
                vxg.append(t)
            y_norm_qs = [
                pqkv.tile([128, 2, 1024], BF16, name=f"yn{qq}")
                for qq in range(2)
            ]

            # ---------------- q/k projections ----------------
            stgv = []
            with (
                tc.tile_pool(name="proj", bufs=1) as pp,
                tc.tile_pool(name="stg", bufs=12) as pstg,
            ):
                dma_engs = [nc.sync, nc.scalar]
                wq = pp.tile([128, 8, MPC], BF16)
                # split so the kc=0 matmuls can start after ~256KB; the
                # rest rides behind the kc=1 stage
                nc.sync.dma_start(wq[:, 0:2, :], wq_in[:, 0:2, :])
                wk = pp.tile([128, 8, MPC], BF16)

                with tc.tile_pool(name="ppsum", bufs=8, space="PSUM") as pps:
                    # --- q projection: single pass over all 2048 cols ---
                    stgq = []
                    for kc in range(8):
                        stg = pstg.tile([128, L], BF16, tag="stage")
                        if kc == 0:
                            # quarter the first stage across both queues so
                            # the kc=0 n=0 matmul starts ~1us sooner
                            for qtr in range(4):
                                dma_engs[qtr % 2].dma_start(
                                    stg[:, 512 * qtr : 512 * qtr + 512],
                                    qT_in[0:128, 512 * qtr : 512 * qtr + 512],
                                )
                        else:
                            dma_engs[(kc + 1) % 2].dma_start(
                                stg[:], qT_in[128 * kc : 128 * kc + 128, :]
                            )
                        stgq.append(stg)
                        if kc == 1:
                            nc.sync.dma_start(wq[:, 2:8, :], wq_in[:, 2:8, :])
                        if kc == 3:
                            nc.scalar.dma_start(wk[:], wk_in[:])
                        if kc == 7:
                            nc.scalar.dma_start(bq[:], bq_in[:])
                            nc.scalar.dma_start(bk[:], bk_in[:])
                    psums = [
                        pps.tile([128, 512], F32, tag="qk", name=f"qkp{i}")
                        for i in range(8)
                    ]
                    for kc in range(8):
                        for m in range(2):
                            for n in range(4):
                                nc.tensor.matmul(
                                    psums[m * 4 + n][:],
                                    wq[:, kc, 128 * m : 128 * m + 128],
                                    stgq[kc][:, 512 * n : 512 * n + 512],
                                    start=(kc == 0),
                                    stop=(kc == 7),
                                )
                    for m in range(2):
                        for n in range(4):
                            for sub in range(2):
                                pb = 64 * sub
                                nc.vector.tensor_scalar_add(
                                    qTz[2 * m + sub][
                                        pb : pb + 64,
                                        512 * n : 512 * n + 512,
                                    ],
                                    psums[m * 4 + n][pb : pb + 64, :],
                                    bq[pb : pb + 64, m : m + 1],
                                )

                    # --- k projection: two column-half passes, so kT cols
                    # 0-1023 (all qs0 needs) are ready half a projection
                    # early and qs0 scores/exp start sooner ---
                    stgk = []
                    for kc in range(8):
                        stg = pstg.tile([128, L], BF16, tag="stage")
                        dma_engs[kc % 2].dma_start(
                            stg[:], kT_in[128 * kc : 128 * kc + 128, :]
                        )
                        stgk.append(stg)
                        if kc == 5:
                            for hh in range(HPC):
                                nc.scalar.dma_start(
                                    eband_t[hh][:], eband_in[hh]
                                )
                    psk = [
                        pps.tile([128, 512], F32, tag="qk", name=f"kp{i}")
                        for i in range(4)
                    ]
                    for kc in range(8):
                        for m in range(2):
                            for nb in range(2):
                                nc.tensor.matmul(
                                    psk[m * 2 + nb][:],
                                    wk[:, kc, 128 * m : 128 * m + 128],
                                    stgk[kc][:, 512 * nb : 512 * nb + 512],
                                    start=(kc == 0),
                                    stop=(kc == 7),
                                )
                    for m in range(2):
                        for nb in range(2):
                            nc.vector.tensor_scalar_add(
                                kTt[m][:, 512 * nb : 512 * nb + 512],
                                psk[m * 2 + nb][:],
                                bk[:, m : m + 1],
                            )

                    # v stages: loaded here (ring has room once q frees), v
                    # matmuls run in the attention phase below.
                    nc.scalar.dma_start(wv[:], wv_in[:])
                    for kc in range(8):
                        s = pstg.tile([128, L], BF16, tag="stage")
                        dma_engs[kc % 2].dma_start(
                            s[:], vT_in[128 * kc : 128 * kc + 128, :]
                        )
                        stgv.append(s)
                        if kc == 3:
                            nc.sync.dma_start(wo[:], wo_in[:])

                # ---------------- attention + out-projection ----------------
                with (
                    tc.tile_pool(name="es", bufs=26) as pes,
                    tc.tile_pool(name="misc", bufs=2) as pmisc,
                    tc.tile_pool(name="ost", bufs=4) as post,
                    tc.tile_pool(name="spsum", bufs=2, space="PSUM") as psc,
                    tc.tile_pool(name="ypsum", bufs=1, space="PSUM") as psy,
                    tc.tile_pool(name="opsum", bufs=2, space="PSUM") as pso,
                ):
                    def emit_vgrp_pair(grp, pair):
                        # 2 l-blocks at a time on the outproj psum ring
                        pv = [
                            pso.tile([128, 512], F32, tag="out",
                                     name=f"vp{grp}{pair}{i}")
                            for i in range(2)
                        ]
                        for kc in range(8):
                            for i in range(2):
                                li = 4 * grp + 2 * pair + i
                                nc.tensor.matmul(
                                    pv[i][:, 0:256],
                                    stgv[kc][:, 128 * li : 128 * li + 128],
                                    wv[:, kc, :],
                                    start=(kc == 0),
                                    stop=(kc == 7),
                                )
                        for i in range(2):
                            li = 4 * grp + 2 * pair + i
                            nc.vector.tensor_copy(
                                vxg[grp][:, 2 * pair + i, :, 0:HD],
                                pv[i][:, 0:256].rearrange(
                                    "p (h d) -> p h d", h=HPC
                                ),
                            )

                    pending_norm = [None]

                    def _emit_norm(item):
                        rrow, pb, mt, qsi = item
                        if qsi == 1:
                            # PE replication: keeps the tail off the gpsimd
                            # ring (busy with output DMAs + broadcasts there)
                            for half in range(2):
                                hof = 512 * half
                                rep = pso.tile([64, 512], F32, tag="out")
                                nc.tensor.matmul(
                                    rep[:],
                                    ones_v[:],
                                    rrow[:, hof : hof + 512],
                                    start=True,
                                    stop=True,
                                )
                                nc.vector.tensor_tensor(
                                    y_norm_qs[qsi][
                                        pb : pb + 64, mt, hof : hof + 512
                                    ],
                                    y_norm_qs[qsi][
                                        pb : pb + 64, mt, hof : hof + 512
                                    ],
                                    rep[:],
                                    MUL,
                                )
                            return
                        prep_sb = pmisc.tile([128, 1024], BF16, tag="prep")
                        nc.gpsimd.partition_broadcast(prep_sb[:], rrow[:])
                        nc.vector.tensor_tensor(
                            y_norm_qs[qsi][pb : pb + 64, mt, :],
                            y_norm_qs[qsi][pb : pb + 64, mt, :],
                            prep_sb[pb : pb + 64, :],
                            MUL,
                        )

                    def _emit_outproj(qsi, qhs=(0, 1)):
                        paired = len(qhs) == 2
                        for n in range(8):
                            ost2 = (
                                post.tile(
                                    [128, 1024], BF16, tag="ost2", name="ost2"
                                )
                                if paired
                                else None
                            )
                            for qh in qhs:
                                qoff = 512 * qh
                                qi = 2 * qsi + qh
                                po = pso.tile([128, 512], F32, tag="out")
                                for c in range(2):
                                    nc.tensor.matmul(
                                        po[:],
                                        wo[:, c, 128 * n : 128 * n + 128],
                                        y_norm_qs[qsi][:, c, qoff : qoff + 512],
                                        start=(c == 0),
                                        stop=(c == 1),
                                    )
                                if paired:
                                    dst = ost2[:, 512 * qh : 512 * qh + 512]
                                else:
                                    dst = post.tile(
                                        [128, 512], BF16, tag="ost", name="ost"
                                    )
                                if qsi == 1 and (n + qh) % 2 == 1:
                                    nc.scalar.copy(dst, po[:])
                                else:
                                    nc.vector.tensor_copy(dst, po[:])
                                if not paired:
                                    [nc.sync, nc.gpsimd][n % 2].dma_start(
                                        outT[
                                            128 * n : 128 * n + 128,
                                            512 * qi : 512 * qi + 512,
                                        ],
                                        dst,
                                    )
                            if paired:
                                # tail (qsi=1) DMAs ride the ACT ring (idle
                                # once exp is done); gpsimd serves qsi=0
                                eng2 = nc.scalar if qsi == 1 else nc.gpsimd
                                [nc.sync, eng2][n % 2].dma_start(
                                    outT[
                                        128 * n : 128 * n + 128,
                                        1024 * qsi : 1024 * qsi + 1024,
                                    ],
                                    ost2[:],
                                )

                    def emit_section(qs, h, defer_av=False, filler=None):
                        q0 = 1024 * qs
                        n_live = 8 * (qs + 1)
                        mt = h // 2
                        pb = 64 * (h % 2)
                        last = (qs, h) == (1, HPC - 1)
                        live_half = [
                            min(4 * (2 * qs + j + 1), 16) for j in (0, 1)
                        ]
                        yT = psy.tile([HD + 1, 1024], F32, tag="yT")
                        if last and pending_norm[0] is not None:
                            _emit_norm(pending_norm[0])
                            pending_norm[0] = None

                        def _emit_av(pend):
                            pes_t, pspecs, pki = pend
                            for j, sj in pspecs:
                                nc.tensor.matmul(
                                    yT[:, 512 * j + sj : 512 * j + 512],
                                    vxg[pki // 4][:, pki % 4, h, :],
                                    pes_t[:, 512 * j + sj : 512 * j + 512],
                                    start=(pki == 0),
                                    stop=(pki == live_half[j] - 1),
                                )

                        deferred = []
                        pending = None
                        for ki in range(n_live):
                            base = 128 * ki - q0
                            s = max(0, base)
                            specs = [
                                (j, max(0, base - 512 * j))
                                for j in (0, 1)
                                if ki < live_half[j]
                            ]
                            sp = psc.tile([128, 1024], F32, tag="sc",
                                          name="sp")
                            for j, sj in specs:
                                c0 = 512 * j + sj
                                nc.tensor.matmul(
                                    sp[:, c0 : 512 * j + 512],
                                    kTt[mt][:, 128 * ki : 128 * ki + 128],
                                    qTz[h][:, q0 + c0 : q0 + 512 * j + 512],
                                    start=True,
                                    stop=True,
                                )
                            es = pes.tile([128, 1024], BF16, tag="es")
                            nc.scalar.activation(
                                es[:, s:1024], sp[:, s:1024], Exp
                            )
                            e = min(1024, base + BW)
                            if e > s:
                                u0 = s - base
                                nc.vector.tensor_tensor(
                                    es[:, s:e],
                                    es[:, s:e],
                                    eband_t[h][:, u0 : u0 + (e - s)],
                                    MUL,
                                )
                            if defer_av:
                                deferred.append((es, specs, ki))
                            else:
                                if pending is not None:
                                    _emit_av(pending)
                                pending = (es, specs, ki)
                            if filler is not None and filler:
                                filler.pop(0)()
                        if defer_av:
                            def flush():
                                for pend in deferred:
                                    _emit_av(pend)
                                _section_tail(qs, h, mt, pb, last, yT)
                            return flush
                        _emit_av(pending)
                        _section_tail(qs, h, mt, pb, last, yT)
                        return None

                    def _section_tail(qs, h, mt, pb, last, yT):
                        if not last:
                            # yT evac FIRST (releases the single yT buffer),
                            # then the recip chain; the replication +
                            # in-place multiply for the PREVIOUS section is
                            # emitted now (its rrow is long ready).
                            nc.vector.tensor_copy(
                                y_norm_qs[qs][pb : pb + 64, mt, :],
                                yT[0:HD, :],
                            )
                            dcp = pmisc.tile([1, 1024], F32, tag="dcp")
                            nc.vector.tensor_copy(dcp[:], yT[HD : HD + 1, :])
                            dT = pmisc.tile([128, 8], F32, tag="dT")
                            nc.sync.dma_start(dT[:], dcp[:])
                            rT = pmisc.tile([128, 8], BF16, tag="rT")
                            with nc.allow_low_precision(
                                reason="softmax recip bf16"
                            ):
                                nc.vector.reciprocal(rT[:], dT[:])
                            rrow = pmisc.tile([1, 1024], BF16, tag="rrow")
                            nc.sync.dma_start(rrow[:], rT[:])
                            if pending_norm[0] is not None:
                                _emit_norm(pending_norm[0])
                            pending_norm[0] = (rrow, pb, mt, qs)
                        else:
                            # final section: lean chain — one custom-DVE
                            # reciprocal straight off the PSUM denominator
                            # row, then fp32 PE replication.
                            # denominator chain on ACT (idle now) so the DVE
                            # yT evacuation doesn't delay the reciprocal
                            dcp = pmisc.tile([1, 1024], F32, tag="dcp")
                            nc.scalar.copy(dcp[:], yT[HD : HD + 1, :])
                            rrec = pmisc.tile([1, 1024], F32, tag="rrec")
                            nc.vector.reciprocal_approx_fast(rrec[:], dcp[:])
                            rrec_b = pmisc.tile([1, 1024], BF16, tag="rrecb")
                            with nc.allow_low_precision(
                                reason="softmax recip bf16"
                            ):
                                nc.scalar.copy(rrec_b[:], rrec[:])
                            nc.vector.tensor_copy(
                                y_norm_qs[qs][pb : pb + 64, mt, :],
                                yT[0:HD, :],
                            )
                            _emit_outproj(0, (1,))
                            for half in range(2):
                                hof = 512 * half
                                rep = pso.tile([64, 512], F32, tag="out")
                                nc.tensor.matmul(
                                    rep[:],
                                    ones_v[:],
                                    rrec_b[:, hof : hof + 512],
                                    start=True,
                                    stop=True,
                                )
                                nc.vector.tensor_tensor(
                                    y_norm_qs[qs][
                                        pb : pb + 64, mt, hof : hof + 512
                                    ],
                                    y_norm_qs[qs][
                                        pb : pb + 64, mt, hof : hof + 512
                                    ],
                                    rep[:],
                                    MUL,
                                )
                        if (qs, h) == (1, 0):
                            _emit_outproj(0, (0,))

                    # qs0 sections (0,0)-(0,2) emit scores+exp only (AVs
                    # deferred); between score blocks the PE runs "filler"
                    # chunks — k pass B and the v groups — so ACT saturates
                    # on exp from the moment k cols 0-1023 exist while the
                    # PE never blocks on the 2-deep sp ring.
                    pkb = [
                        psc.tile([128, 1024], F32, tag="sc", name=f"kb{m}")
                        for m in range(2)
                    ]

                    def _kb_chunk(kc):
                        # k pass B: cols 1024-2047 on psc half-tiles (each
                        # 512-f32 half is its own 2KB psum zero-region)
                        def f():
                            for m in range(2):
                                for nb in range(2):
                                    nc.tensor.matmul(
                                        pkb[m][:, 512 * nb : 512 * nb + 512],
                                        wk[:, kc, 128 * m : 128 * m + 128],
                                        stgk[kc][
                                            :,
                                            1024 + 512 * nb :
                                            1024 + 512 * nb + 512,
                                        ],
                                        start=(kc == 0),
                                        stop=(kc == 7),
                                    )
                        return f

                    def _kb_evac():
                        for m in range(2):
                            nc.vector.tensor_scalar_add(
                                kTt[m][:, 1024:2048],
                                pkb[m][:],
                                bk[:, m : m + 1],
                            )

                    def _vgrp_pair(grp, pair):
                        def f():
                            emit_vgrp_pair(grp, pair)
                        return f

                    filler = [_kb_chunk(kc) for kc in range(8)]
                    filler.append(_kb_evac)
                    for grp in range(4):
                        for pair in range(2):
                            filler.append(_vgrp_pair(grp, pair))

                    flushes = []
                    for h in range(3):
                        flushes.append(
                            emit_section(0, h, defer_av=True, filler=filler)
                        )
                    # drain any unconsumed filler (vgrp3 tail)
                    while filler:
                        filler.pop(0)()
                    for fl in flushes:
                        fl()
                    emit_section(0, 3)
                    for h in range(HPC):
                        emit_section(1, h)
                    _emit_outproj(1)

    nc.finalize()
    return nc


def _host_tables(rel_emb: np.ndarray):
    """Per-head multiplicative exp-band stripes [128, BW]; eb[r,u] multiplies
    es for key-row r, query-col (128*ki + u). Zero above the diagonal
    (causal mask), exp(bias - c31) within distance<113, exactly 1 beyond
    (handled by not multiplying outside the stripe)."""
    r = np.arange(128)[:, None]
    u = np.arange(BW)[None, :]
    rp = r - u  # key - query
    buckets = _bucket(rp)
    ebs = []
    for h in range(H):
        c31 = np.float32(rel_emb[31, h])
        vals = rel_emb[buckets, h].astype(np.float32) - c31
        eb = np.where(rp > 0, np.float32(0.0), np.exp(vals))
        ebs.append(eb.astype(NP_BF16))
    return ebs


def _numpy_ref(query, key, value, attn_mask, key_padding_mask,
               Wq, bq, Wk, bk, Wv, bv, Wo, bo, rel_emb):
    """Exact numpy fallback for unexpected mask patterns."""
    q = (query @ Wq.T + bq).reshape(B, L, H, HD).transpose(0, 2, 1, 3)
    k = (key @ Wk.T + bk).reshape(B, L, H, HD).transpose(0, 2, 1, 3)
    v = (value @ Wv.T + bv).reshape(B, L, H, HD).transpose(0, 2, 1, 3)
    scores = np.einsum("bhqd,bhkd->bhqk", q, k) / math.sqrt(HD)
    rp = np.arange(L, dtype=np.int64)[None, :] - np.arange(L, dtype=np.int64)[:, None]
    rel = rel_emb[_bucket(rp)].transpose(2, 0, 1)
    scores = scores + rel[None]
    scores = np.where(attn_mask[None, None], scores, -np.inf)
    scores = np.where(key_padding_mask[:, None, None, :], scores, -np.inf)
    scores = scores - scores.max(-1, keepdims=True)
    e = np.exp(scores)
    attn = e / e.sum(-1, keepdims=True)
    out = np.einsum("bhqk,bhkd->bhqd", attn, v)
    out = out.transpose(0, 2, 1, 3).reshape(B, L, D)
    return (out @ Wo.T + bo).astype(np.float32)


def _in_maps(inp):
    query, key, value = inp["query"], inp["key"], inp["value"]
    Wq, bq, Wk, bk = inp["Wq"], inp["bq"], inp["Wk"], inp["bk"]
    Wv, Wo = inp["Wv"], inp["Wo"]
    rel_emb = inp["rel_emb"]

    ebands = _host_tables(rel_emb)

    def _rearr_w(w_slice):  # [MPC, D] row-major weights -> [128, 8, MPC]
        arr = np.ascontiguousarray(w_slice.T)  # [D, MPC]
        return arr.reshape(8, 128, MPC).transpose(1, 0, 2).astype(NP_BF16)

    in_maps = []
    for c in range(N_CORES):
        b, hg = c // HPC, c % HPC
        rows = slice(MPC * hg, MPC * hg + MPC)
        heads = range(HPC * hg, HPC * hg + HPC)
        wo_c = np.ascontiguousarray(Wo[:, rows].T)  # [MPC, D]
        in_maps.append({
            "qT_in": query[b].T.astype(NP_BF16),
            "kT_in": key[b].T.astype(NP_BF16),
            "vT_in": value[b].T.astype(NP_BF16),
            "wq_in": _rearr_w(Wq[rows] / math.sqrt(HD)),
            "wk_in": _rearr_w(Wk[rows]),
            "wv_in": _rearr_w(Wv[rows]),
            "wo_in": wo_c.reshape(2, 128, D).transpose(1, 0, 2).astype(NP_BF16),
            "bq_in": np.ascontiguousarray(
                (bq[rows] / math.sqrt(HD)).reshape(2, 128).T.astype(np.float32)
            ),
            "bk_in": np.ascontiguousarray(
                bk[rows].reshape(2, 128).T.astype(np.float32)
            ),
            "eband_in": np.stack([ebands[h] for h in heads]),
        })
    return in_maps


def kernel(**inputs) -> np.ndarray:
    global _cached, last_results
    inp = {k: np.asarray(v) for k, v in inputs.items()}
    attn_mask, kpm = inp["attn_mask"], inp["key_padding_mask"]

    causal = np.array_equal(attn_mask, np.tril(np.ones((L, L), bool)))
    if not (causal and kpm.all()):
        return _numpy_ref(**inp)

    if _cached is None:
        _cached = _build()
    nc = _cached

    res = run_bass_kernel_spmd(nc, _in_maps(inp), list(range(N_CORES)))
    last_results = res

    bo, bv, Wo = inp["bo"], inp["bv"], inp["Wo"]
    bo_eff = (
        bo.astype(np.float64) + bv.astype(np.float64) @ Wo.T.astype(np.float64)
    )
    out = np.empty((B, L, D), np.float32)
    for b in range(B):
        acc = np.zeros((D, L), np.float64)
        for hg in range(HPC):
            acc += res.results[b * HPC + hg]["outT"].astype(np.float64)
        out[b] = (acc.T + bo_eff[None, :]).astype(np.float32)
    return out


# revision 25
# speedup vs baseline: 1.0402x; 1.0270x over previous
"""Bass/Trainium2 kernel for nn_MultiHeadAttention (T5-style rel-bias causal MHA).

Sharding: 8 cores = 2 batches x 4 head-groups (4 heads of 64 dims each).
Each core: projects q/k/v for its 256 proj rows, runs causal attention, and
computes a partial out-projection. Host sums the 4 partials per batch.

v5 vs v3 (191.6us baseline):
- The T5 relative bias + causal mask fold into a multiplicative exp(band)
  table applied to es on the DVE over a 240-wide near-diagonal stripe
  (exp(s+b) = exp(s)*exp(b); masked positions multiply by 0). This removes
  all PE band-preload matmuls and the const-block bookkeeping; every score
  matmul is a single start/stop K=128 matmul.
- v-projection runs inside the attention phase on the out-projection PSUM
  pool, its 4 l-block groups interleaved with the qs0 attention sections:
  qs0's exp backlog drains on ACT while the PE runs v matmuls. qs0 AV only
  needs v groups 0-1.
- Output DMAs ride the gpsimd ring instead of the ACT ring (ACT paces the
  attention phase via exp).
"""
import math
import sys

sys.path.insert(0, "/opt/trn_rl_repo")

import ml_dtypes
import numpy as np

from concourse import bacc
import concourse.mybir as mybir
import concourse.tile as tile
from concourse.bass_utils import run_bass_kernel_spmd

F32 = mybir.dt.float32
BF16 = mybir.dt.bfloat16
Exp = mybir.ActivationFunctionType.Exp
MUL = mybir.AluOpType.mult
NP_BF16 = ml_dtypes.bfloat16

B, L, D = 2, 2048, 1024
H, HD = 16, 64
NUM_BUCKETS, MAX_DISTANCE = 32, 128
HPC = 4  # heads per core
MPC = HPC * HD  # 256 proj rows per core
N_CORES = 8
BW = 240  # exp-band stripe width (bias==c31 and unmasked beyond it)

last_results = None  # BassKernelResults of the most recent run (for profiling)
_cached = None


def _bucket(rp: np.ndarray) -> np.ndarray:
    """T5 relative position bucket, mirrors the reference exactly."""
    sign = (rp > 0).astype(np.int32)
    n = np.abs(rp)
    max_exact = NUM_BUCKETS // 2
    n_safe = np.maximum(n, 1).astype(np.float32)
    vil = max_exact + (
        np.log(n_safe / max_exact)
        / math.log(MAX_DISTANCE / max_exact)
        * (NUM_BUCKETS - max_exact)
    ).astype(np.int32)
    vil = np.minimum(vil, NUM_BUCKETS - 1)
    buckets = np.where(n < max_exact, n, vil) + sign * max_exact
    return np.clip(buckets, 0, NUM_BUCKETS - 1)


def _build():
    nc = bacc.Bacc(trn_type="TRN2")

    qT_in = nc.dram_tensor("qT_in", [D, L], BF16, kind="ExternalInput")
    kT_in = nc.dram_tensor("kT_in", [D, L], BF16, kind="ExternalInput")
    vT_in = nc.dram_tensor("vT_in", [D, L], BF16, kind="ExternalInput")
    wq_in = nc.dram_tensor("wq_in", [128, 8, MPC], BF16, kind="ExternalInput")
    wk_in = nc.dram_tensor("wk_in", [128, 8, MPC], BF16, kind="ExternalInput")
    wv_in = nc.dram_tensor("wv_in", [128, 8, MPC], BF16, kind="ExternalInput")
    wo_in = nc.dram_tensor("wo_in", [128, 2, D], BF16, kind="ExternalInput")
    bq_in = nc.dram_tensor("bq_in", [128, 2], F32, kind="ExternalInput")
    bk_in = nc.dram_tensor("bk_in", [128, 2], F32, kind="ExternalInput")
    eband_in = nc.dram_tensor("eband_in", [HPC, 128, BW], BF16,
                              kind="ExternalInput")
    outT = nc.dram_tensor("outT", [D, L], BF16, kind="ExternalOutput")

    with tile.TileContext(nc) as tc:
        with (
            tc.tile_pool(name="res", bufs=1) as pr,
            tc.tile_pool(name="qkv", bufs=1) as pqkv,
        ):
            eband_t = [
                pqkv.tile([128, BW], BF16, name=f"eb{hh}") for hh in range(HPC)
            ]
            wo = pqkv.tile([128, 2, D], BF16)
            wv = pqkv.tile([128, 8, MPC], BF16)

            bq = pr.tile([128, 2], F32)
            bk = pr.tile([128, 2], F32)
            # warm the ACT exp table early, off the critical path
            warm = pr.tile([1, 2], F32)
            nc.vector.memset(warm[:], 0.0)
            nc.scalar.activation(warm[:], warm[:], Exp)
            ones_v = pr.tile([1, HD], BF16)
            nc.vector.memset(ones_v[:], 1.0)

            qTz = []
            for hh in range(HPC):
                t = pqkv.tile([128, L], BF16, name=f"qtz{hh}")
                nc.vector.memset(t[:].bitcast(F32), 0.0)
                qTz.append(t)
            kTt = [
                pqkv.tile([128, L], BF16, name=f"kt{mm}") for mm in range(2)
            ]
            vxg = []
            for g in range(4):
                t = pqkv.tile([128, 4, HPC, HD + 1], BF16, name=f"vx{g}")
                nc.vector.memset(t[:, :, :, HD], 1.0)
                vxg.append(t)
            y_norm_qs = [
                pqkv.tile([128, 2, 1024], BF16, name=f"yn{qq}")
                for qq in range(2)
            ]

            # ---------------- q/k projections ----------------
            stgv = []
            with (
                tc.tile_pool(name="proj", bufs=1) as pp,
                tc.tile_pool(name="stg", bufs=16) as pstg,
            ):
                dma_engs = [nc.sync, nc.scalar]
                wq = pp.tile([128, 8, MPC], BF16)
                # split so the kc=0 matmuls can start after ~256KB; the
                # rest rides behind the kc=1 stage
                nc.sync.dma_start(wq[:, 0:2, :], wq_in[:, 0:2, :])
                wk = pp.tile([128, 8, MPC], BF16)

                with tc.tile_pool(name="ppsum", bufs=8, space="PSUM") as pps:
                    # --- q projection: single pass over all 2048 cols ---
                    stgq = []
                    for kc in range(8):
                        stg = pstg.tile([128, L], BF16, tag="stage")
                        if kc == 0:
                            # quarter the first stage across both queues so
                            # the kc=0 n=0 matmul starts ~1us sooner
                            for qtr in range(4):
                                dma_engs[qtr % 2].dma_start(
                                    stg[:, 512 * qtr : 512 * qtr + 512],
                                    qT_in[0:128, 512 * qtr : 512 * qtr + 512],
                                )
                        else:
                            dma_engs[(kc + 1) % 2].dma_start(
                                stg[:], qT_in[128 * kc : 128 * kc + 128, :]
                            )
                        stgq.append(stg)
                        if kc == 1:
                            nc.sync.dma_start(wq[:, 2:8, :], wq_in[:, 2:8, :])
                        if kc == 3:
                            nc.scalar.dma_start(wk[:], wk_in[:])
                        if kc == 7:
                            nc.scalar.dma_start(bq[:], bq_in[:])
                            nc.scalar.dma_start(bk[:], bk_in[:])
                    psums = [
                        pps.tile([128, 512], F32, tag="qk", name=f"qkp{i}")
                        for i in range(8)
                    ]
                    for kc in range(8):
                        for m in range(2):
                            for n in range(4):
                                nc.tensor.matmul(
                                    psums[m * 4 + n][:],
                                    wq[:, kc, 128 * m : 128 * m + 128],
                                    stgq[kc][:, 512 * n : 512 * n + 512],
                                    start=(kc == 0),
                                    stop=(kc == 7),
                                )
                    for m in range(2):
                        for n in range(4):
                            for sub in range(2):
                                pb = 64 * sub
                                nc.vector.tensor_scalar_add(
                                    qTz[2 * m + sub][
                                        pb : pb + 64,
                                        512 * n : 512 * n + 512,
                                    ],
                                    psums[m * 4 + n][pb : pb + 64, :],
                                    bq[pb : pb + 64, m : m + 1],
                                )

                    # --- k projection: two column-half passes, so kT cols
                    # 0-1023 (all qs0 needs) are ready half a projection
                    # early and qs0 scores/exp start sooner ---
                    stgk = []
                    for kc in range(8):
                        stg = pstg.tile([128, L], BF16, tag="stage")
                        dma_engs[kc % 2].dma_start(
                            stg[:], kT_in[128 * kc : 128 * kc + 128, :]
                        )
                        stgk.append(stg)
                        if kc == 5:
                            for hh in range(HPC):
                                nc.scalar.dma_start(
                                    eband_t[hh][:], eband_in[hh]
                                )
                    psk = [
                        pps.tile([128, 512], F32, tag="qk", name=f"kp{i}")
                        for i in range(8)
                    ]
                    for kc in range(8):
                        for m in range(2):
                            for n in range(4):
                                nc.tensor.matmul(
                                    psk[m * 4 + n][:],
                                    wk[:, kc, 128 * m : 128 * m + 128],
                                    stgk[kc][:, 512 * n : 512 * n + 512],
                                    start=(kc == 0),
                                    stop=(kc == 7),
                                )
                    for m in range(2):
                        for n in range(4):
                            nc.vector.tensor_scalar_add(
                                kTt[m][:, 512 * n : 512 * n + 512],
                                psk[m * 4 + n][:],
                                bk[:, m : m + 1],
                            )

                    # v stages: loaded here (ring has room once q frees), v
                    # matmuls run in the attention phase below.
                    nc.scalar.dma_start(wv[:], wv_in[:])
                    for kc in range(8):
                        s = pstg.tile([128, L], BF16, tag="stage")
                        dma_engs[kc % 2].dma_start(
                            s[:], vT_in[128 * kc : 128 * kc + 128, :]
                        )
                        stgv.append(s)
                        if kc == 3:
                            nc.sync.dma_start(wo[:], wo_in[:])

                # ---------------- attention + out-projection ----------------
                with (
                    tc.tile_pool(name="es", bufs=12) as pes,
                    tc.tile_pool(name="misc", bufs=3) as pmisc,
                    tc.tile_pool(name="ost", bufs=4) as post,
                    tc.tile_pool(name="spsum", bufs=2, space="PSUM") as psc,
                    tc.tile_pool(name="ypsum", bufs=1, space="PSUM") as psy,
                    tc.tile_pool(name="opsum", bufs=2, space="PSUM") as pso,
                ):
                    def emit_vgrp_pair(grp, pair):
                        # 2 l-blocks at a time on the outproj psum ring
                        pv = [
                            pso.tile([128, 512], F32, tag="out",
                                     name=f"vp{grp}{pair}{i}")
                            for i in range(2)
                        ]
                        for kc in range(8):
                            for i in range(2):
                                li = 4 * grp + 2 * pair + i
                                nc.tensor.matmul(
                                    pv[i][:, 0:256],
                                    stgv[kc][:, 128 * li : 128 * li + 128],
                                    wv[:, kc, :],
                                    start=(kc == 0),
                                    stop=(kc == 7),
                                )
                        for i in range(2):
                            li = 4 * grp + 2 * pair + i
                            nc.vector.tensor_copy(
                                vxg[grp][:, 2 * pair + i, :, 0:HD],
                                pv[i][:, 0:256].rearrange(
                                    "p (h d) -> p h d", h=HPC
                                ),
                            )

                    pending_norm = [None]

                    def _emit_norm(item):
                        rrow, pb, mt, qsi = item
                        prep_sb = pmisc.tile([128, 1024], BF16, tag="prep")
                        nc.gpsimd.partition_broadcast(prep_sb[:], rrow[:])
                        nc.vector.tensor_tensor(
                            y_norm_qs[qsi][pb : pb + 64, mt, :],
                            y_norm_qs[qsi][pb : pb + 64, mt, :],
                            prep_sb[pb : pb + 64, :],
                            MUL,
                        )

                    def _emit_outproj(qsi, qhs=(0, 1)):
                        paired = len(qhs) == 2
                        for n in range(8):
                            ost2 = (
                                post.tile(
                                    [128, 1024], BF16, tag="ost2", name="ost2"
                                )
                                if paired
                                else None
                            )
                            for qh in qhs:
                                qoff = 512 * qh
                                qi = 2 * qsi + qh
                                po = pso.tile([128, 512], F32, tag="out")
                                for c in range(2):
                                    nc.tensor.matmul(
                                        po[:],
                                        wo[:, c, 128 * n : 128 * n + 128],
                                        y_norm_qs[qsi][:, c, qoff : qoff + 512],
                                        start=(c == 0),
                                        stop=(c == 1),
                                    )
                                if paired:
                                    dst = ost2[:, 512 * qh : 512 * qh + 512]
                                else:
                                    dst = post.tile(
                                        [128, 512], BF16, tag="ost", name="ost"
                                    )
                                if qsi == 1 and (n + qh) % 2 == 1:
                                    nc.scalar.copy(dst, po[:])
                                else:
                                    nc.vector.tensor_copy(dst, po[:])
                                if not paired:
                                    [nc.sync, nc.gpsimd][n % 2].dma_start(
                                        outT[
                                            128 * n : 128 * n + 128,
                                            512 * qi : 512 * qi + 512,
                                        ],
                                        dst,
                                    )
                            if paired:
                                [nc.sync, nc.gpsimd][n % 2].dma_start(
                                    outT[
                                        128 * n : 128 * n + 128,
                                        1024 * qsi : 1024 * qsi + 1024,
                                    ],
                                    ost2[:],
                                )

                    def emit_section(qs, h, defer_av=False, filler=None):
                        q0 = 1024 * qs
                        n_live = 8 * (qs + 1)
                        mt = h // 2
                        pb = 64 * (h % 2)
                        last = (qs, h) == (1, HPC - 1)
                        live_half = [
                            min(4 * (2 * qs + j + 1), 16) for j in (0, 1)
                        ]
                        yT = psy.tile([HD + 1, 1024], F32, tag="yT")
                        if last and pending_norm[0] is not None:
                            _emit_norm(pending_norm[0])
                            pending_norm[0] = None

                        def _emit_av(pend):
                            pes_t, pspecs, pki = pend
                            for j, sj in pspecs:
                                nc.tensor.matmul(
                                    yT[:, 512 * j + sj : 512 * j + 512],
                                    vxg[pki // 4][:, pki % 4, h, :],
                                    pes_t[:, 512 * j + sj : 512 * j + 512],
                                    start=(pki == 0),
                                    stop=(pki == live_half[j] - 1),
                                )

                        deferred = []
                        pending = None
                        for ki in range(n_live):
                            base = 128 * ki - q0
                            s = max(0, base)
                            specs = [
                                (j, max(0, base - 512 * j))
                                for j in (0, 1)
                                if ki < live_half[j]
                            ]
                            sp = psc.tile([128, 1024], F32, tag="sc",
                                          name="sp")
                            for j, sj in specs:
                                c0 = 512 * j + sj
                                nc.tensor.matmul(
                                    sp[:, c0 : 512 * j + 512],
                                    kTt[mt][:, 128 * ki : 128 * ki + 128],
                                    qTz[h][:, q0 + c0 : q0 + 512 * j + 512],
                                    start=True,
                                    stop=True,
                                )
                            es = pes.tile([128, 1024], BF16, tag="es")
                            nc.scalar.activation(
                                es[:, s:1024], sp[:, s:1024], Exp
                            )
                            e = min(1024, base + BW)
                            if e > s:
                                u0 = s - base
                                nc.vector.tensor_tensor(
                                    es[:, s:e],
                                    es[:, s:e],
                                    eband_t[h][:, u0 : u0 + (e - s)],
                                    MUL,
                                )
                            if defer_av:
                                deferred.append((es, specs, ki))
                            else:
                                if pending is not None:
                                    _emit_av(pending)
                                pending = (es, specs, ki)
                            if filler is not None and filler:
                                filler.pop(0)()
                        if defer_av:
                            def flush():
                                for pend in deferred:
                                    _emit_av(pend)
                                _section_tail(qs, h, mt, pb, last, yT)
                            return flush
                        _emit_av(pending)
                        _section_tail(qs, h, mt, pb, last, yT)
                        return None

                    def _section_tail(qs, h, mt, pb, last, yT):
                        if not last:
                            # yT evac FIRST (releases the single yT buffer),
                            # then the recip chain; the replication +
                            # in-place multiply for the PREVIOUS section is
                            # emitted now (its rrow is long ready).
                            nc.vector.tensor_copy(
                                y_norm_qs[qs][pb : pb + 64, mt, :],
                                yT[0:HD, :],
                            )
                            dcp = pmisc.tile([1, 1024], F32, tag="dcp")
                            nc.vector.tensor_copy(dcp[:], yT[HD : HD + 1, :])
                            dT = pmisc.tile([128, 8], F32, tag="dT")
                            nc.sync.dma_start(dT[:], dcp[:])
                            rT = pmisc.tile([128, 8], BF16, tag="rT")
                            with nc.allow_low_precision(
                                reason="softmax recip bf16"
                            ):
                                nc.vector.reciprocal(rT[:], dT[:])
                            rrow = pmisc.tile([1, 1024], BF16, tag="rrow")
                            nc.sync.dma_start(rrow[:], rT[:])
                            if pending_norm[0] is not None:
                                _emit_norm(pending_norm[0])
                            pending_norm[0] = (rrow, pb, mt, qs)
                        else:
                            # final section: lean chain — one custom-DVE
                            # reciprocal straight off the PSUM denominator
                            # row, then fp32 PE replication.
                            dcp = pmisc.tile([1, 1024], F32, tag="dcp")
                            nc.vector.tensor_copy(dcp[:], yT[HD : HD + 1, :])
                            rrec = pmisc.tile([1, 1024], F32, tag="rrec")
                            nc.vector.reciprocal_approx_fast(rrec[:], dcp[:])
                            rrec_b = pmisc.tile([1, 1024], BF16, tag="rrecb")
                            with nc.allow_low_precision(
                                reason="softmax recip bf16"
                            ):
                                nc.vector.tensor_copy(rrec_b[:], rrec[:])
                            nc.vector.tensor_copy(
                                y_norm_qs[qs][pb : pb + 64, mt, :],
                                yT[0:HD, :],
                            )
                            _emit_outproj(0, (1,))
                            for half in range(2):
                                hof = 512 * half
                                rep = pso.tile([64, 512], F32, tag="out")
                                nc.tensor.matmul(
                                    rep[:],
                                    ones_v[:],
                                    rrec_b[:, hof : hof + 512],
                                    start=True,
                                    stop=True,
                                )
                                nc.vector.tensor_tensor(
                                    y_norm_qs[qs][
                                        pb : pb + 64, mt, hof : hof + 512
                                    ],
                                    y_norm_qs[qs][
                                        pb : pb + 64, mt, hof : hof + 512
                                    ],
                                    rep[:],
                                    MUL,
                                )
                        if (qs, h) == (1, 0):
                            _emit_outproj(0, (0,))

                    # Section (0,0)'s scores+exp run BEFORE the v groups
                    # (exp needs only q/k); its AVs flush after groups 0-1
                    # land. Remaining v groups interleave with qs0 sections
                    # so their matmuls run while exp drains on ACT.
                    flush00 = emit_section(0, 0, defer_av=True)
                    emit_vgrp_pair(0, 0)
                    emit_vgrp_pair(0, 1)
                    emit_vgrp_pair(1, 0)
                    emit_vgrp_pair(1, 1)
                    flush00()
                    emit_section(0, 1)
                    emit_vgrp_pair(2, 0)
                    emit_vgrp_pair(2, 1)
                    emit_section(0, 2)
                    emit_vgrp_pair(3, 0)
                    emit_vgrp_pair(3, 1)
                    emit_section(0, 3)
                    for h in range(HPC):
                        emit_section(1, h)
                    _emit_outproj(1)

    nc.finalize()
    return nc


def _host_tables(rel_emb: np.ndarray):
    """Per-head multiplicative exp-band stripes [128, BW]; eb[r,u] multiplies
    es for key-row r, query-col (128*ki + u). Zero above the diagonal
    (causal mask), exp(bias - c31) within distance<113, exactly 1 beyond
    (handled by not multiplying outside the stripe)."""
    r = np.arange(128)[:, None]
    u = np.arange(BW)[None, :]
    rp = r - u  # key - query
    buckets = _bucket(rp)
    ebs = []
    for h in range(H):
        c31 = np.float32(rel_emb[31, h])
        vals = rel_emb[buckets, h].astype(np.float32) - c31
        eb = np.where(rp > 0, np.float32(0.0), np.exp(vals))
        ebs.append(eb.astype(NP_BF16))
    return ebs


def _numpy_ref(query, key, value, attn_mask, key_padding_mask,
               Wq, bq, Wk, bk, Wv, bv, Wo, bo, rel_emb):
    """Exact numpy fallback for unexpected mask patterns."""
    q = (query @ Wq.T + bq).reshape(B, L, H, HD).transpose(0, 2, 1, 3)
    k = (key @ Wk.T + bk).reshape(B, L, H, HD).transpose(0, 2, 1, 3)
    v = (value @ Wv.T + bv).reshape(B, L, H, HD).transpose(0, 2, 1, 3)
    scores = np.einsum("bhqd,bhkd->bhqk", q, k) / math.sqrt(HD)
    rp = np.arange(L, dtype=np.int64)[None, :] - np.arange(L, dtype=np.int64)[:, None]
    rel = rel_emb[_bucket(rp)].transpose(2, 0, 1)
    scores = scores + rel[None]
    scores = np.where(attn_mask[None, None], scores, -np.inf)
    scores = np.where(key_padding_mask[:, None, None, :], scores, -np.inf)
    scores = scores - scores.max(-1, keepdims=True)
    e = np.exp(scores)
    attn = e / e.sum(-1, keepdims=True)
    out = np.einsum("bhqk,bhkd->bhqd", attn, v)
    out = out.transpose(0, 2, 1, 3).reshape(B, L, D)
    return (out @ Wo.T + bo).astype(np.float32)


def _in_maps(inp):
    query, key, value = inp["query"], inp["key"], inp["value"]
    Wq, bq, Wk, bk = inp["Wq"], inp["bq"], inp["Wk"], inp["bk"]
    Wv, Wo = inp["Wv"], inp["Wo"]
    rel_emb = inp["rel_emb"]

    ebands = _host_tables(rel_emb)

    def _rearr_w(w_slice):  # [MPC, D] row-major weights -> [128, 8, MPC]
        arr = np.ascontiguousarray(w_slice.T)  # [D, MPC]
        return arr.reshape(8, 128, MPC).transpose(1, 0, 2).astype(NP_BF16)

    in_maps = []
    for c in range(N_CORES):
        b, hg = c // HPC, c % HPC
        rows = slice(MPC * hg, MPC * hg + MPC)
        heads = range(HPC * hg, HPC * hg + HPC)
        wo_c = np.ascontiguousarray(Wo[:, rows].T)  # [MPC, D]
        in_maps.append({
            "qT_in": query[b].T.astype(NP_BF16),
            "kT_in": key[b].T.astype(NP_BF16),
            "vT_in": value[b].T.astype(NP_BF16),
            "wq_in": _rearr_w(Wq[rows] / math.sqrt(HD)),
            "wk_in": _rearr_w(Wk[rows]),
            "wv_in": _rearr_w(Wv[rows]),
            "wo_in": wo_c.reshape(2, 128, D).transpose(1, 0, 2).astype(NP_BF16),
            "bq_in": np.ascontiguousarray(
                (bq[rows] / math.sqrt(HD)).reshape(2, 128).T.astype(np.float32)
            ),
            "bk_in": np.ascontiguousarray(
                bk[rows].reshape(2, 128).T.astype(np.float32)
            ),
            "eband_in": np.stack([ebands[h] for h in heads]),
        })
    return in_maps


def kernel(**inputs) -> np.ndarray:
    global _cached, last_results
    inp = {k: np.asarray(v) for k, v in inputs.items()}
    attn_mask, kpm = inp["attn_mask"], inp["key_padding_mask"]

    causal = np.array_equal(attn_mask, np.tril(np.ones((L, L), bool)))
    if not (causal and kpm.all()):
        return _numpy_ref(**inp)

    if _cached is None:
        _cached = _build()
    nc = _cached

    res = run_bass_kernel_spmd(nc, _in_maps(inp), list(range(N_CORES)))
    last_results = res

    bo, bv, Wo = inp["bo"], inp["bv"], inp["Wo"]
    bo_eff = (
        bo.astype(np.float64) + bv.astype(np.float64) @ Wo.T.astype(np.float64)
    )
    out = np.empty((B, L, D), np.float32)
    for b in range(B):
        acc = np.zeros((D, L), np.float64)
        for hg in range(HPC):
            acc += res.results[b * HPC + hg]["outT"].astype(np.float64)
        out[b] = (acc.T + bo_eff[None, :]).astype(np.float32)
    return out
